# revision 6
# baseline (speedup 1.0000x reference)
"""Bass/Trainium2 kernel for nn_CurveGraphic2d (min-distance curve rasterizer).

kernel(**inputs) takes FULL inputs (inputs [64,4,2] f32, widths [64] f32,
aa_factors [64] f32) and returns the FULL [64,256,256] float32 canvas.

Math (per curve b, output element [b, i, j]; reference flattens its pixel
grid x-major, so row index i is the x coordinate, column j is y):

    md2    = min_s (j - sy_bs)^2 + (i - sx_bs)^2
    canvas = clip(1 - (md2/w_b^2)^(aa_b/2), 0, 1)

Softmin-by-matmul: with k_b = C / w_b^2,

    p(i,j)  = sum_s exp(-k d2_s) = sum_s exp(-k (i-sx_s)^2) * exp(-k (j-sy_s)^2)
            = sum_s U[s,i] * V[s,j]                    (rank-15 outer product)
    md2 ~= -ln(p) / k_b        (softmin; error ~ -ln(n_eff)/k, |.| <= 2.7 w^2/C)

so the whole distance field + min collapses into one tiny bf16 matmul per
(curve, x-half): lhsT = U [15,128], rhs = V [15,256], PSUM out [128,256].
C = 86 puts the fp32/bf16 exp underflow horizon (e^-87.3) right at the
clip boundary md = w: far pixels flush to p = 0 -> ln -> -inf -> canvas 0,
exactly matching the reference's clipped zeros.  Host-side numpy sim of the
full dtype pipeline (bf16 U/V, fp32 PSUM w/ FTZ, fp16 tail) measures global
rel L2 error 2.7e-3 vs the fp64 reference (tolerance 2e-2).

Tail over the full per-core canvas [128, 4096] (8 curves x 2 halves x 256):
    t  = Ln(p)                        ACT, PSUM -> SBUF fp16
    t2 = min(t, -0.002)               DVE (guards Ln(<=0) -> NaN near curve
                                       where p > 1 i.e. softmin z < 0)
    s2 = Ln(-t2 / C)                  ACT   ( = ln(z/C) = ln(md2/w^2) )
    u  = s2 * (aa/2)                  DVE, per-curve tile operand
    r  = Exp(u)                       ACT   ( = (md/w)^aa )
    o  = max(1 - r, 0)                DVE x2
Ln+Exp share one ACT table set (natural_log_exp_and_others).  ACT is the
steady-state bottleneck: 3 passes x (4096+352)/1.2GHz ~= 11.1 us.

Device decomposition: data-parallel over curves, core c owns curves
[8c, 8c+8); no cross-core communication.  Output fp16, cast on host.
"""

import numpy as np
from math import comb

H = W = 256
S = 15
B = 64
NCORES = 8
CPB = B // NCORES          # curves per core
UNITS = CPB * 2            # (curve, x-half) units per core

C_SOFT = 86.0              # softmin sharpness: k = C/w^2
FLUSH = 1.1755e-38         # fp32/bf16 min normal: pre-flush denormal U/V

# The hardware Ln spline is exact only on [2^-64, 2^64] (saturates below,
# garbage above).  p spans [2^-126, 2^4], so Ln#1 prescales by 2^61:
# t = Ln(p * 2^61) = ln p + 61 ln 2, keeping inputs in [2^-65.2, 2^65).
# The offset folds into Ln#2's bias.  z = -ln p = LN_OFF - t.
LN_SCALE_E = 61
LN_OFF = LN_SCALE_E * float(np.log(2.0))      # 42.2804
T_CLAMP = 42.25            # fp16-exact; z >= LN_OFF - T_CLAMP = 0.0304 > 0

_prog_cache = {}


# ---------------------------------------------------------------------------
# host-side math
# ---------------------------------------------------------------------------

def _bezier_samples(inputs_np):
    """[B,S,2] float64 sample points (y, x) in pixel coords."""
    kp = inputs_np.astype(np.float64) * np.array([H, W], np.float64)
    K = kp.shape[1]
    ts = np.linspace(0.0, 1.0, S)
    k = np.arange(K)
    binom = np.array([comb(K - 1, i) for i in range(K)], np.float64)
    basis = binom * ts[:, None] ** k * (1.0 - ts[:, None]) ** (K - 1 - k)
    return np.einsum("sk,bkd->bsd", basis, kp)


def _make_core_inputs(sp, widths, aas, core):
    """Input tensors for one core (curves [8*core, 8*core+8))."""
    import ml_dtypes

    bf16 = ml_dtypes.bfloat16
    coords = np.arange(256, dtype=np.float64)
    ut = np.zeros((S, UNITS * 128), np.float32)   # exp(-k (i-sx)^2), x-major
    vt = np.zeros((S, CPB * 256), np.float32)     # exp(-k (j-sy)^2)
    at = np.zeros((128, UNITS * 256), np.float16)  # aa/2 per curve block
    with np.errstate(under="ignore"):
        for cl in range(CPB):
            b = core * CPB + cl
            kb = C_SOFT / float(widths[b]) ** 2
            sy, sx = sp[b, :, 0], sp[b, :, 1]
            U = np.exp(-kb * (coords[None, :] - sx[:, None]) ** 2)  # [S,256]
            V = np.exp(-kb * (coords[None, :] - sy[:, None]) ** 2)  # [S,256]
            for h in range(2):
                ut[:, (cl * 2 + h) * 128:(cl * 2 + h + 1) * 128] = \
                    U[:, h * 128:(h + 1) * 128]
            vt[:, cl * 256:(cl + 1) * 256] = V
            at[:, cl * 512:(cl + 1) * 512] = np.float16(aas[b] / 2.0)
    ut[ut < FLUSH] = 0.0
    vt[vt < FLUSH] = 0.0
    utb = ut.astype(bf16)
    vtb = vt.astype(bf16)
    # flush any bf16 denormals produced by the rounding itself
    utb[utb.astype(np.float32) < FLUSH] = 0
    vtb[vtb.astype(np.float32) < FLUSH] = 0
    return {"ut": utb, "vt": vtb, "at": at}


# ---------------------------------------------------------------------------
# multi-wait workaround
# ---------------------------------------------------------------------------

def _split_multi_waits(nc):
    """This walrus build accepts only one sync-wait per instruction.  Hoist
    extra waits onto same-engine nops inserted just before the instruction
    (engine program order makes this semantically identical: all waits retire
    before the instruction issues)."""
    import concourse.mybir as mybir

    n = 0
    for fn in nc.m.functions:
        for bb in fn.blocks:
            insts = list(bb.instructions)
            out = []
            changed = False
            for inst in insts:
                si = inst.sync_info
                if si is not None and len(si.on_wait) > 1:
                    waits = list(si.on_wait)
                    for i, w in enumerate(waits[:-1]):
                        nop = mybir.InstNoOp(name=f"{inst.name}_xw{i}")
                        nop.engine = inst.engine
                        nop.sync_info = mybir.SyncInfo(on_wait=[w], on_update=[])
                        out.append(nop)
                        n += 1
                    inst.sync_info = mybir.SyncInfo(
                        on_wait=[waits[-1]], on_update=list(si.on_update)
                    )
                    changed = True
                out.append(inst)
            if changed:
                bb.instructions = out
    return n


# ---------------------------------------------------------------------------
# bass program (input-independent structure)
# ---------------------------------------------------------------------------

def _build_program(repeat=1):
    import concourse.bass as bass
    import concourse.mybir as mybir
    from concourse.tile import TileContext

    fp32 = mybir.dt.float32
    fp16 = mybir.dt.float16
    bf16 = mybir.dt.bfloat16
    A = mybir.AluOpType
    F = mybir.ActivationFunctionType

    nc = bass.Bass("TRN2", target_bir_lowering=False, debug=False,
                   num_devices=NCORES)
    ut_d = nc.dram_tensor("ut", [S, UNITS * 128], bf16, kind="ExternalInput")
    vt_d = nc.dram_tensor("vt", [S, CPB * 256], bf16, kind="ExternalInput")
    at_d = nc.dram_tensor("at", [128, UNITS * 256], fp16, kind="ExternalInput")
    out_d = nc.dram_tensor("out", [128, UNITS * 256], fp16,
                           kind="ExternalOutput")

    with TileContext(nc) as tc:
        with (
            tc.tile_pool(name="const", bufs=1) as constp,
            tc.tile_pool(name="tail", bufs=1) as tailp,
            tc.tile_pool(name="ot", bufs=2) as otp,
            tc.psum_pool(name="psum", bufs=1) as psp,
        ):
            ut = constp.tile([S, UNITS * 128], bf16, tag="ut")
            nc.sync.dma_start(out=ut[:], in_=ut_d[:])
            vt = constp.tile([S, CPB * 256], bf16, tag="vt")
            nc.sync.dma_start(out=vt[:], in_=vt_d[:])
            at = constp.tile([128, UNITS * 256], fp16, tag="at")
            nc.sync.dma_start(out=at[:], in_=at_d[:])

            pp = psp.tile([128, UNITS * 256], fp32, tag="pp")
            t = tailp.tile([128, UNITS * 256], fp32, tag="t")
            t2 = tailp.tile([128, UNITS * 256], fp16, tag="t2")
            s2 = tailp.tile([128, UNITS * 256], fp16, tag="s2")
            uu = tailp.tile([128, UNITS * 256], fp16, tag="uu")
            rr = tailp.tile([128, UNITS * 256], fp16, tag="rr")
            oo = otp.tile([128, UNITS * 256], fp16, tag="oo")

            def body():
                # p = sum_s U[s,i] V[s,j] per (curve, half); PSUM bank = curve
                for cl in range(CPB):
                    for h in range(2):
                        u = cl * 2 + h
                        nc.tensor.matmul(
                            pp[:, u * 256:(u + 1) * 256],
                            ut[:, u * 128:(u + 1) * 128],
                            vt[:, cl * 256:(cl + 1) * 256],
                            start=True, stop=True,
                        )
                # t = ln p + LN_OFF   (fp32: keeps z resolution relative)
                nc.scalar.activation(t[:], pp[:], F.Ln,
                                     scale=float(2.0 ** LN_SCALE_E))
                # t2 = min(t, T_CLAMP) - LN_OFF = -z,  z >= 0.03
                nc.vector.tensor_scalar(t2[:], t[:], T_CLAMP, LN_OFF,
                                        A.min, A.subtract)
                # s2 = ln(-t2 / C) = ln(z/C) = ln(md2 / w^2)
                nc.scalar.activation(s2[:], t2[:], F.Ln, scale=-1.0 / C_SOFT)
                # u = (aa/2) ln(md2/w^2)
                nc.vector.tensor_tensor(uu[:], s2[:], at[:], A.mult)
                # r = (md/w)^aa
                nc.scalar.activation(rr[:], uu[:], F.Exp)
                # o = max(1 - r, 0)
                nc.vector.tensor_scalar(oo[:], rr[:], -1.0, 1.0, A.mult, A.add)
                nc.vector.tensor_scalar_max(oo[:], oo[:], 0.0)
                nc.sync.dma_start(out=out_d[:], in_=oo[:])

            for _ in range(repeat):
                body()
    _split_multi_waits(nc)
    return nc


# ---------------------------------------------------------------------------
# public entry point
# ---------------------------------------------------------------------------

def _run(inputs, widths, aa_factors, repeat=1):
    from concourse.bass_utils import run_bass_kernel_spmd

    inputs = np.asarray(inputs, np.float32)
    widths = np.asarray(widths, np.float32)
    aa_factors = np.asarray(aa_factors, np.float32)
    assert inputs.shape == (B, 4, 2), inputs.shape

    sp = _bezier_samples(inputs)
    if repeat not in _prog_cache:
        _prog_cache[repeat] = _build_program(repeat)
    nc = _prog_cache[repeat]

    in_maps = [
        _make_core_inputs(sp, widths, aa_factors, c) for c in range(NCORES)
    ]
    res = run_bass_kernel_spmd(nc, in_maps, list(range(NCORES)))

    canvas = np.empty((B, H, W), np.float32)
    for c in range(NCORES):
        out = np.asarray(res.results[c]["out"])          # [128, 4096] fp16
        out = out.reshape(128, CPB, 2, 256)              # [i, cl, h, y]
        out = out.transpose(1, 2, 0, 3)                  # [cl, h, i, y]
        canvas[c * CPB:(c + 1) * CPB] = out.reshape(CPB, 256, 256)
    return canvas


def kernel(inputs, widths, aa_factors):
    return _run(inputs, widths, aa_factors, repeat=1)
